# revision 6
# baseline (speedup 1.0000x reference)
"""Trainium2 Bass kernel for nn_Evolution (snake polygon evolution, 3 rounds).

Strategy:
  - Data-parallel over polygons: 256 polys -> 8 cores x 32 polys, processed in
    8 groups of 4 polys (N=512 matmul free dim = 4 polys x 128 points).
  - cnn_feature is pre-transposed on host to a [B*H*W, C] bf16 row table,
    replicated to all cores. Bilinear sampling = 2 indirect-DMA strip gathers
    per polygon (one per y-corner row; each descriptor grabs the 2 x-adjacent
    pixels = 128 contiguous bf16), combined on DVE with per-point weights.
    X-edge clipping uses a weight-swap trick so a strip is always in-bounds.
  - Snake runs channel-on-partition: circular conv = 9 shifted matmuls
    (taps) accumulating in PSUM, on a circularly-padded activation buffer.
    BatchNorm scale is folded into conv weights on host (biases are zero).
    relu+residual is a single fused scalar_tensor_tensor per layer.
  - All 3 evolve rounds run in ONE kernel launch. Evolve-1 sample offsets/
    weights are host-precomputed; evolve-2/3 offsets/weights are computed on
    device from the previous round's output polygons.
"""

import numpy as np
import ml_dtypes

import concourse.bass as bass
import concourse.bacc as bacc
import concourse.mybir as mybir
import concourse.tile as tile
from concourse.bass_utils import run_bass_kernel_spmd

F32 = mybir.dt.float32
BF16 = mybir.dt.bfloat16
I32 = mybir.dt.int32
OP = mybir.AluOpType

B, C, H, W = 16, 64, 128, 128
N, P = 256, 128
NCORES = 8
NPC = N // NCORES          # polys per core = 32
GP = 4                     # polys per group
G = NPC // GP              # groups per core = 8
NFREE = GP * P             # 512
SD = 2 * C                 # state_dim = 128
FD = C + 2                 # feature_dim = 66
PAD = 16                   # circular pad (max dilation 4 * 4 taps)
SEG = P + 2 * PAD          # 160 per-poly segment in padded buffers
DILS = [1, 1, 1, 2, 2, 4, 4]
RO = 4.0
BN_EPS = 1e-5
HWPIX = H * W

_CACHE = {}


# ---------------------------------------------------------------- device code

def _emit_pads(nc, xp, nrows):
    """Fill circular wrap regions of a padded [nrows, G?*SEG] activation tile
    (one group's 4 polys)."""
    v = xp[:nrows].rearrange("p (g s) -> p g s", g=GP)
    # left pad [0,PAD) <- body last PAD cols = [P, P+PAD) of body = cols [P+PAD-PAD...]
    nc.vector.tensor_copy(v[:, :, 0:PAD], v[:, :, P : P + PAD])
    # right pad [PAD+P, SEG) <- body first PAD = cols [PAD, 2*PAD)
    nc.vector.tensor_copy(v[:, :, PAD + P : SEG], v[:, :, PAD : 2 * PAD])


def _conv_taps(nc, psum, lhsT_tile, xp, nrows, dil, ntaps=9):
    """Accumulate 9 shifted matmuls into psum [SD, NFREE]."""
    for k in range(ntaps):
        off = PAD + (k - ntaps // 2) * dil
        rhs = xp[:nrows].rearrange("p (g s) -> p g s", g=GP)[:, :, off : off + P]
        nc.tensor.matmul(
            out=psum[:],
            lhsT=lhsT_tile[:, k, :],
            rhs=rhs,
            start=(k == 0),
            stop=(k == ntaps - 1),
        )


def _build_program():
    nc = bacc.Bacc("TRN2", target_bir_lowering=False, debug=False)

    feat = nc.dram_tensor("feat", [B * HWPIX, C], BF16, kind="ExternalInput")
    offs1 = nc.dram_tensor("offs1", [G, P, 8], I32, kind="ExternalInput")
    wts1 = nc.dram_tensor("wts1", [G, P, 16], F32, kind="ExternalInput")
    iitro1 = nc.dram_tensor("iitro1", [G, 2, NFREE], F32, kind="ExternalInput")
    rocan1 = nc.dram_tensor("rocan1", [G, 2, NFREE], BF16, kind="ExternalInput")
    bhw = nc.dram_tensor("bhw", [P, NPC], F32, kind="ExternalInput")
    wdr = {}
    for s in range(3):
        wdr[s] = {
            "head": nc.dram_tensor(f"w{s}_head", [FD, 9 * SD], BF16, kind="ExternalInput"),
            "res": [
                nc.dram_tensor(f"w{s}_res{i}", [SD, 9 * SD], BF16, kind="ExternalInput")
                for i in range(7)
            ],
            "fus": nc.dram_tensor(f"w{s}_fus", [SD, 8 * 256], BF16, kind="ExternalInput"),
            "p0g": nc.dram_tensor(f"w{s}_p0g", [SD, 2 * 256], BF16, kind="ExternalInput"),
            "p0s": nc.dram_tensor(f"w{s}_p0s", [SD, 8 * 256], BF16, kind="ExternalInput"),
            "p1": nc.dram_tensor(f"w{s}_p1", [SD, 2 * 64], BF16, kind="ExternalInput"),
            "p2": nc.dram_tensor(f"w{s}_p2", [64, 2], BF16, kind="ExternalInput"),
        }
    pyout = nc.dram_tensor("pyout", [3, G, P, 8], F32, kind="ExternalOutput")

    with tile.TileContext(nc) as tc:
        with (
            tc.tile_pool(name="wp", bufs=1) as wp,
            tc.tile_pool(name="persist", bufs=2 * G) as pers,
            tc.tile_pool(name="work", bufs=3) as wk,
            tc.tile_pool(name="state", bufs=2) as sp,
            tc.tile_pool(name="prep", bufs=2) as pp,
            tc.tile_pool(name="psum", bufs=2, space="PSUM") as ps,
        ):
            # ---- constants / weights (loaded once)
            from concourse.masks import make_identity

            ident = wp.tile([P, P], F32)
            make_identity(nc, ident[:])

            bhw_t = wp.tile([P, NPC], F32)
            nc.sync.dma_start(bhw_t[:], bhw[:])

            wt = {}
            for s in range(3):
                d = wdr[s]
                wt[s] = {
                    "head": wp.tile([FD, 9, SD], BF16, tag=f"w{s}head", name=f"w{s}head"),
                    "res": [wp.tile([SD, 9, SD], BF16, tag=f"w{s}res{i}", name=f"w{s}res{i}") for i in range(7)],
                    "fus": wp.tile([SD, 8, 256], BF16, tag=f"w{s}fus", name=f"w{s}fus"),
                    "p0g": wp.tile([SD, 2, 256], BF16, tag=f"w{s}p0g", name=f"w{s}p0g"),
                    "p0s": wp.tile([SD, 8, 256], BF16, tag=f"w{s}p0s", name=f"w{s}p0s"),
                    "p1": wp.tile([SD, 2, 64], BF16, tag=f"w{s}p1", name=f"w{s}p1"),
                    "p2": wp.tile([64, 2], BF16, tag=f"w{s}p2", name=f"w{s}p2"),
                }
                nc.sync.dma_start(wt[s]["head"][:].rearrange("a b c -> a (b c)"), d["head"][:])
                for i in range(7):
                    nc.sync.dma_start(wt[s]["res"][i][:].rearrange("a b c -> a (b c)"), d["res"][i][:])
                nc.sync.dma_start(wt[s]["fus"][:].rearrange("a b c -> a (b c)"), d["fus"][:])
                nc.sync.dma_start(wt[s]["p0g"][:].rearrange("a b c -> a (b c)"), d["p0g"][:])
                nc.sync.dma_start(wt[s]["p0s"][:].rearrange("a b c -> a (b c)"), d["p0s"][:])
                nc.sync.dma_start(wt[s]["p1"][:].rearrange("a b c -> a (b c)"), d["p1"][:])
                nc.sync.dma_start(wt[s]["p2"][:], d["p2"][:])

            # persistent per-group tiles carried between evolves
            offs_cur = [None] * G
            wts_cur = [None] * G
            rocan_cur = [None] * G
            py_cur = [None] * G

            for ev in range(3):
                w = wt[ev]
                for g in range(G):
                    # ---------- sampling inputs
                    if ev == 0:
                        offs_t = pers.tile([P, 8], I32, tag="offs")
                        wts_t = pers.tile([P, 16], F32, tag="wts")
                        nc.sync.dma_start(offs_t[:], offs1[g])
                        nc.sync.dma_start(wts_t[:], wts1[g])
                    else:
                        offs_t = offs_cur[g]
                        wts_t = wts_cur[g]

                    # ---------- gather strips: [P, poly, strip(y0/y1), 2C]
                    gath = wk.tile([P, GP, 2, 2 * C], BF16, tag="gath")
                    for j in range(GP):
                        for s in range(2):
                            nc.gpsimd.indirect_dma_start(
                                out=gath[:, j, s, :],
                                out_offset=None,
                                in_=feat[:],
                                in_offset=bass.IndirectOffsetOnAxis(
                                    ap=offs_t[:, s * GP + j : s * GP + j + 1], axis=0
                                ),
                            )

                    # ---------- bilinear combine -> comb [P, (poly, C)] f32
                    def cslice(si, xi):  # strip si, x-slot xi
                        return gath[:, :, si, xi * C : (xi + 1) * C]

                    def wbc(ci):  # corner weight broadcast [P, GP, C]
                        return wts_t[:, ci * GP : (ci + 1) * GP].to_broadcast([P, GP, C])

                    m0 = wk.tile([P, GP, C], F32, tag="m0")
                    m1 = wk.tile([P, GP, C], F32, tag="m1")
                    comb = wk.tile([P, GP, C], F32, tag="comb")
                    nc.vector.tensor_tensor(out=m0[:], in0=cslice(0, 0), in1=wbc(0), op=OP.mult)
                    nc.vector.tensor_tensor(out=m1[:], in0=cslice(0, 1), in1=wbc(1), op=OP.mult)
                    nc.vector.tensor_add(out=m0[:], in0=m0[:], in1=m1[:])
                    nc.vector.tensor_tensor(out=m1[:], in0=cslice(1, 0), in1=wbc(2), op=OP.mult)
                    nc.vector.tensor_add(out=m0[:], in0=m0[:], in1=m1[:])
                    nc.vector.tensor_tensor(out=m1[:], in0=cslice(1, 1), in1=wbc(3), op=OP.mult)
                    nc.vector.tensor_add(out=comb[:], in0=m0[:], in1=m1[:])

                    # ---------- build inp [FD, GP*SEG] bf16 (points transposed back)
                    inp = wk.tile([FD, GP * SEG], BF16, tag="inp")
                    inpv = inp[:].rearrange("a (g s) -> a g s", g=GP)
                    for j in range(GP):
                        tp = ps.tile([C, P], F32, tag="tp", space="PSUM")
                        nc.tensor.transpose(out=tp[:], in_=comb[:, j, :], identity=ident[:])
                        nc.vector.tensor_copy(out=inpv[0:C, j, PAD : PAD + P], in_=tp[:])
                    # rows 64:66 = c_it * RO (channel-major)
                    dst = inpv[C:FD, :, PAD : PAD + P]
                    if ev == 0:
                        rc = wk.tile([2, NFREE], BF16, tag="rc")
                        nc.sync.dma_start(rc[:], rocan1[g])
                        nc.vector.tensor_copy(out=dst, in_=rc[:].rearrange("a (g p) -> a g p", g=GP))
                    else:
                        nc.vector.tensor_copy(
                            out=dst,
                            in_=rocan_cur[g][:].rearrange("a (g p) -> a g p", g=GP),
                        )
                    _emit_pads(nc, inp, FD)

                    # ---------- snake: head + 7 res blocks
                    states = []
                    s0 = sp.tile([SD, GP * SEG], BF16, tag="s0")
                    pc = ps.tile([SD, NFREE], F32, tag="conv", space="PSUM")
                    _conv_taps(nc, pc, w["head"], inp, FD, 1)
                    s0v = s0[:].rearrange("a (g s) -> a g s", g=GP)
                    nc.vector.tensor_scalar(
                        out=s0v[:, :, PAD : PAD + P],
                        in0=pc[:].rearrange("a (g p) -> a g p", g=GP),
                        scalar1=0.0, scalar2=None, op0=OP.max,
                    )
                    _emit_pads(nc, s0, SD)
                    states.append(s0)
                    xprev = s0
                    for i, d in enumerate(DILS):
                        si = sp.tile([SD, GP * SEG], BF16, tag=f"s{i + 1}")
                        pc = ps.tile([SD, NFREE], F32, tag="conv", space="PSUM")
                        _conv_taps(nc, pc, w["res"][i], xprev, SD, d)
                        body = lambda t: t[:].rearrange("a (g s) -> a g s", g=GP)[
                            :, :, PAD : PAD + P
                        ]
                        # s_{i+1} = relu(conv) + s_i  (fused)
                        nc.vector.scalar_tensor_tensor(
                            out=body(si),
                            in0=pc[:].rearrange("a (g p) -> a g p", g=GP),
                            scalar=0.0, in1=body(xprev),
                            op0=OP.max, op1=OP.add,
                        )
                        _emit_pads(nc, si, SD)
                        states.append(si)
                        xprev = si

                    # ---------- fusion conv1x1 + global max -> glob [SD, 2*GP] bf16
                    glob = wk.tile([SD, 2 * GP], BF16, tag="glob")
                    globf = wk.tile([SD, 2 * GP], F32, tag="globf")
                    for m in range(2):
                        pf = ps.tile([SD, NFREE], F32, tag="big2", space="PSUM")
                        for c in range(8):
                            rhs = states[c][:].rearrange("a (g s) -> a g s", g=GP)[
                                :, :, PAD : PAD + P
                            ]
                            nc.tensor.matmul(
                                out=pf[:], lhsT=w["fus"][:, c, m * SD : (m + 1) * SD],
                                rhs=rhs, start=(c == 0), stop=(c == 7),
                            )
                        nc.vector.tensor_reduce(
                            out=globf[:, m * GP : (m + 1) * GP],
                            in_=pf[:].rearrange("a (g p) -> a g p", g=GP),
                            axis=mybir.AxisListType.X, op=OP.max,
                        )
                    nc.vector.tensor_copy(out=glob[:], in_=globf[:])

                    # ---------- p0: relu(W_g @ glob_bcast + W_s @ state)
                    h0 = []
                    for m in range(2):
                        pp0 = ps.tile([SD, NFREE], F32, tag="big2", space="PSUM")
                        for hh in range(2):
                            nc.tensor.matmul(
                                out=pp0[:],
                                lhsT=w["p0g"][:, hh, m * SD : (m + 1) * SD],
                                rhs=glob[:, hh * GP : (hh + 1) * GP].to_broadcast([SD, GP, P]),
                                start=(hh == 0), stop=False,
                            )
                        for c in range(8):
                            rhs = states[c][:].rearrange("a (g s) -> a g s", g=GP)[
                                :, :, PAD : PAD + P
                            ]
                            nc.tensor.matmul(
                                out=pp0[:], lhsT=w["p0s"][:, c, m * SD : (m + 1) * SD],
                                rhs=rhs, start=False, stop=(c == 7),
                            )
                        hm = wk.tile([SD, NFREE], BF16, tag=f"h0{m}")
                        nc.vector.tensor_scalar(
                            out=hm[:], in0=pp0[:], scalar1=0.0, scalar2=None, op0=OP.max
                        )
                        h0.append(hm)

                    # ---------- p1: relu(W @ h0) -> h1 [64, NFREE] bf16
                    pp1 = ps.tile([64, NFREE], F32, tag="small", space="PSUM")
                    for m in range(2):
                        nc.tensor.matmul(
                            out=pp1[:], lhsT=w["p1"][:, m, :], rhs=h0[m][:],
                            start=(m == 0), stop=(m == 1),
                        )
                    h1 = wk.tile([64, NFREE], BF16, tag="h1")
                    nc.vector.tensor_scalar(
                        out=h1[:], in0=pp1[:], scalar1=0.0, scalar2=None, op0=OP.max
                    )

                    # ---------- p2 -> off [2, NFREE]; py = i_it*RO + off
                    pp2 = ps.tile([2, NFREE], F32, tag="small", space="PSUM")
                    nc.tensor.matmul(out=pp2[:], lhsT=w["p2"][:], rhs=h1[:], start=True, stop=True)
                    py = pers.tile([2, NFREE], F32, tag="py")
                    if ev == 0:
                        iit = wk.tile([2, NFREE], F32, tag="iit")
                        nc.sync.dma_start(iit[:], iitro1[g])
                        nc.vector.tensor_add(out=py[:], in0=pp2[:], in1=iit[:])
                    else:
                        nc.vector.tensor_add(out=py[:], in0=pp2[:], in1=py_cur[g][:])
                    py_cur[g] = py

                    # ---------- transpose py -> pyt [P, (poly,2)] , DMA out
                    ptp = ps.tile([P, 8], F32, tag="small", space="PSUM")
                    for j in range(GP):
                        nc.tensor.transpose(
                            out=ptp[:, 2 * j : 2 * j + 2],
                            in_=py[:, j * P : (j + 1) * P],
                            identity=ident[0:2, 0:2],
                        )
                    pyt = wk.tile([P, 8], F32, tag="pyt")
                    nc.vector.tensor_copy(out=pyt[:], in_=ptp[:])
                    nc.sync.dma_start(pyout[ev, g], pyt[:])

                    # ---------- prep next-evolve sampling (offsets + weights)
                    if ev < 2:
                        TT = lambda tag: pp.tile([P, 8], F32, tag=tag, name=tag)
                        T4 = lambda tag: pp.tile([P, GP], F32, tag=tag, name=tag)
                        a = TT("a")
                        # a = py/4 - 0.5 + 1024
                        nc.vector.tensor_scalar(
                            out=a[:], in0=pyt[:], scalar1=0.25, scalar2=1023.5,
                            op0=OP.mult, op1=OP.add,
                        )
                        ai = pp.tile([P, 8], I32, tag="ai")
                        nc.vector.tensor_copy(out=ai[:], in_=a[:])
                        af = TT("af")
                        nc.vector.tensor_copy(out=af[:], in_=ai[:])
                        gt = TT("gt")
                        nc.vector.tensor_tensor(out=gt[:], in0=af[:], in1=a[:], op=OP.is_gt)
                        flp = TT("flp")
                        nc.vector.tensor_sub(out=flp[:], in0=af[:], in1=gt[:])
                        frac = TT("frac")
                        nc.vector.tensor_sub(out=frac[:], in0=a[:], in1=flp[:])
                        c0 = TT("c0")
                        nc.vector.tensor_scalar(
                            out=c0[:], in0=flp[:], scalar1=1024.0, scalar2=None, op0=OP.subtract
                        )
                        v0 = TT("v0")
                        v1 = TT("v1")
                        t0 = TT("t0")
                        nc.vector.tensor_scalar(out=v0[:], in0=c0[:], scalar1=0.0, scalar2=None, op0=OP.is_ge)
                        nc.vector.tensor_scalar(out=t0[:], in0=c0[:], scalar1=float(W - 1), scalar2=None, op0=OP.is_le)
                        nc.vector.tensor_mul(out=v0[:], in0=v0[:], in1=t0[:])
                        nc.vector.tensor_scalar(out=v1[:], in0=c0[:], scalar1=-1.0, scalar2=None, op0=OP.is_ge)
                        nc.vector.tensor_scalar(out=t0[:], in0=c0[:], scalar1=float(W - 2), scalar2=None, op0=OP.is_le)
                        nc.vector.tensor_mul(out=v1[:], in0=v1[:], in1=t0[:])
                        w1v = TT("w1v")
                        nc.vector.tensor_mul(out=w1v[:], in0=frac[:], in1=v1[:])
                        w0v = TT("w0v")
                        nc.vector.tensor_mul(out=w0v[:], in0=frac[:], in1=v0[:])
                        nc.vector.tensor_sub(out=w0v[:], in0=v0[:], in1=w0v[:])
                        # x-specific (even cols) / y-specific (odd cols) views
                        xv = lambda t: t[:].rearrange("p (j two) -> p j two", two=2)[:, :, 0]
                        yv = lambda t: t[:].rearrange("p (j two) -> p j two", two=2)[:, :, 1]
                        sL = T4("sL"); sR = T4("sR"); mm = T4("mm")
                        nc.vector.tensor_scalar(out=sL[:], in0=xv(c0), scalar1=0.0, scalar2=None, op0=OP.is_lt)
                        nc.vector.tensor_scalar(out=sR[:], in0=xv(c0), scalar1=float(W - 2), scalar2=None, op0=OP.is_gt)
                        nc.vector.tensor_add(out=mm[:], in0=sL[:], in1=sR[:])
                        wA = T4("wA"); wB = T4("wB"); u = T4("u")
                        nc.vector.tensor_mul(out=u[:], in0=xv(w0v), in1=mm[:])
                        nc.vector.tensor_sub(out=wA[:], in0=xv(w0v), in1=u[:])
                        nc.vector.tensor_mul(out=u[:], in0=xv(w1v), in1=sL[:])
                        nc.vector.tensor_add(out=wA[:], in0=wA[:], in1=u[:])
                        nc.vector.tensor_mul(out=u[:], in0=xv(w1v), in1=mm[:])
                        nc.vector.tensor_sub(out=wB[:], in0=xv(w1v), in1=u[:])
                        nc.vector.tensor_mul(out=u[:], in0=xv(w0v), in1=sR[:])
                        nc.vector.tensor_add(out=wB[:], in0=wB[:], in1=u[:])
                        # corner weights
                        wts_n = pers.tile([P, 16], F32, tag="wts")
                        nc.vector.tensor_mul(out=wts_n[:, 0:GP], in0=wA[:], in1=yv(w0v))
                        nc.vector.tensor_mul(out=wts_n[:, GP : 2 * GP], in0=wB[:], in1=yv(w0v))
                        nc.vector.tensor_mul(out=wts_n[:, 2 * GP : 3 * GP], in0=wA[:], in1=yv(w1v))
                        nc.vector.tensor_mul(out=wts_n[:, 3 * GP : 4 * GP], in0=wB[:], in1=yv(w1v))
                        # clipped coords + row indices
                        xc = T4("xc"); yc0 = T4("yc0"); yc1 = T4("yc1")
                        nc.vector.tensor_scalar(out=xc[:], in0=xv(c0), scalar1=0.0, scalar2=float(W - 2), op0=OP.max, op1=OP.min)
                        nc.vector.tensor_scalar(out=yc0[:], in0=yv(c0), scalar1=0.0, scalar2=float(H - 1), op0=OP.max, op1=OP.min)
                        nc.vector.tensor_scalar(out=yc1[:], in0=yv(c0), scalar1=1.0, scalar2=0.0, op0=OP.add, op1=OP.max)
                        nc.vector.tensor_scalar(out=yc1[:], in0=yc1[:], scalar1=float(H - 1), scalar2=None, op0=OP.min)
                        bg = bhw_t[:, g * GP : (g + 1) * GP]
                        r0 = T4("r0"); r1 = T4("r1")
                        nc.vector.scalar_tensor_tensor(out=r0[:], in0=yc0[:], scalar=float(W), in1=bg, op0=OP.mult, op1=OP.add)
                        nc.vector.tensor_add(out=r0[:], in0=r0[:], in1=xc[:])
                        nc.vector.scalar_tensor_tensor(out=r1[:], in0=yc1[:], scalar=float(W), in1=bg, op0=OP.mult, op1=OP.add)
                        nc.vector.tensor_add(out=r1[:], in0=r1[:], in1=xc[:])
                        offs_n = pers.tile([P, 8], I32, tag="offs")
                        nc.vector.tensor_copy(out=offs_n[:, 0:GP], in_=r0[:])
                        nc.vector.tensor_copy(out=offs_n[:, GP : 2 * GP], in_=r1[:])
                        # rocan = py - min_p(py)  (bf16)
                        mn = pp.tile([2, GP], F32, tag="mn")
                        nc.vector.tensor_reduce(
                            out=mn[:], in_=py[:].rearrange("a (g p) -> a g p", g=GP),
                            axis=mybir.AxisListType.X, op=OP.min,
                        )
                        rocan_n = pers.tile([2, NFREE], BF16, tag="rocan")
                        nc.vector.tensor_tensor(
                            out=rocan_n[:].rearrange("a (g p) -> a g p", g=GP),
                            in0=py[:].rearrange("a (g p) -> a g p", g=GP),
                            in1=mn[:].to_broadcast([2, GP, P]),
                            op=OP.subtract,
                        )
                        offs_cur[g] = offs_n
                        wts_cur[g] = wts_n
                        rocan_cur[g] = rocan_n

    nc.compile()
    return nc


# ------------------------------------------------------------------ host code

def _np32(x):
    return np.asarray(x, dtype=np.float32)


def _prep_weights(p):
    """Fold BN scale into conv weights; build SBUF-layout bf16 arrays."""
    bf = ml_dtypes.bfloat16
    for k in ["head_b", "head_bt", "fus_b", "p0_b", "p1_b", "p2_b"] + [
        f"res{i}_{suf}" for i in range(7) for suf in ("b", "bt")
    ]:
        assert np.all(_np32(p[k]) == 0.0), f"nonzero bias {k} unsupported"
    out = {}
    s = _np32(p["head_g"]) / np.sqrt(np.float32(1.0) + np.float32(BN_EPS))
    assert np.all(s > 0)
    whead = _np32(p["head_w"]) * s[:, None, None]  # [SD, FD, 9]
    out["head"] = np.ascontiguousarray(whead.transpose(1, 2, 0)).reshape(FD, 9 * SD).astype(bf)
    out["res"] = []
    for i in range(7):
        si = _np32(p[f"res{i}_g"]) / np.sqrt(np.float32(1.0) + np.float32(BN_EPS))
        assert np.all(si > 0)
        wr = _np32(p[f"res{i}_w"]) * si[:, None, None]
        out["res"].append(
            np.ascontiguousarray(wr.transpose(1, 2, 0)).reshape(SD, 9 * SD).astype(bf)
        )
    wfus = _np32(p["fus_w"])  # [256, 1024]
    out["fus"] = np.ascontiguousarray(wfus.T.reshape(8, SD, 256).transpose(1, 0, 2)).reshape(SD, 8 * 256).astype(bf)
    wp0 = _np32(p["p0_w"])  # [256, 1280]
    out["p0g"] = np.ascontiguousarray(wp0[:, :256].T.reshape(2, SD, 256).transpose(1, 0, 2)).reshape(SD, 2 * 256).astype(bf)
    out["p0s"] = np.ascontiguousarray(wp0[:, 256:].T.reshape(8, SD, 256).transpose(1, 0, 2)).reshape(SD, 8 * 256).astype(bf)
    wp1 = _np32(p["p1_w"])  # [64, 256]
    out["p1"] = np.ascontiguousarray(wp1.T.reshape(2, SD, 64).transpose(1, 0, 2)).reshape(SD, 2 * 64).astype(bf)
    out["p2"] = np.ascontiguousarray(_np32(p["p2_w"]).T).astype(bf)  # [64, 2]
    return out


def _sample_prep_host(polys, ind_core):
    """Reference-exact sampling coords -> strip offsets + corner weights.

    polys: [npc, P, 2] f32 (i_it for evolve 1); ind_core: [npc] int32.
    Returns offs [G, P, 8] i32, wts [G, P, 16] f32.
    """
    px, py_ = polys[..., 0], polys[..., 1]
    f32 = np.float32
    gx = px / f32(W / 2.0) - f32(1.0)
    gy = py_ / f32(H / 2.0) - f32(1.0)
    x = ((gx + f32(1.0)) * f32(W) - f32(1.0)) / f32(2.0)
    y = ((gy + f32(1.0)) * f32(H) - f32(1.0)) / f32(2.0)

    def axis_w(v, n):
        v0 = np.floor(v)
        f = v - v0
        valid0 = (v0 >= 0) & (v0 <= n - 1)
        valid1 = (v0 + 1 >= 0) & (v0 + 1 <= n - 1)
        w0 = (f32(1.0) - f) * valid0
        w1 = f * valid1
        return v0, w0, w1

    x0, wx0, wx1 = axis_w(x, W)
    y0, wy0, wy1 = axis_w(y, H)
    sL = x0 < 0
    sR = x0 > W - 2
    m = sL | sR
    wA = np.where(m, np.where(sL, wx1, f32(0.0)), wx0)
    wB = np.where(m, np.where(sR, wx0, f32(0.0)), wx1)
    xc = np.clip(x0, 0, W - 2)
    yc0 = np.clip(y0, 0, H - 1)
    yc1 = np.clip(y0 + 1, 0, H - 1)
    base = (ind_core.astype(np.int64) * HWPIX)[:, None]
    r0 = (base + yc0 * W + xc).astype(np.int32)
    r1 = (base + yc1 * W + xc).astype(np.int32)
    offs = np.zeros((G, P, 8), np.int32)
    wts = np.zeros((G, P, 16), np.float32)
    for g in range(G):
        for j in range(GP):
            n = g * GP + j
            offs[g, :, 0 * GP + j] = r0[n]
            offs[g, :, 1 * GP + j] = r1[n]
            wts[g, :, 0 * GP + j] = (wA * wy0)[n]
            wts[g, :, 1 * GP + j] = (wB * wy0)[n]
            wts[g, :, 2 * GP + j] = (wA * wy1)[n]
            wts[g, :, 3 * GP + j] = (wB * wy1)[n]
    return offs, wts


def _prepare_in_maps(cnn_feature, img_init_polys, can_init_polys, params, ind):
    bf = ml_dtypes.bfloat16
    feat_np = np.ascontiguousarray(cnn_feature.transpose(0, 2, 3, 1)).reshape(
        B * HWPIX, C
    ).astype(bf)
    wts_by_snake = [_prep_weights(params[f"snake{s}"]) for s in range(3)]

    poly1 = np.stack(
        [
            np.clip(img_init_polys[..., 0], 0.0, np.float32(W - 1.0)),
            np.clip(img_init_polys[..., 1], 0.0, np.float32(H - 1.0)),
        ],
        axis=-1,
    ).astype(np.float32)

    in_maps = []
    for core in range(NCORES):
        sl = slice(core * NPC, (core + 1) * NPC)
        pc = poly1[sl]          # [NPC, P, 2]
        cc = can_init_polys[sl]
        ic = ind[sl]
        offs1, wts1 = _sample_prep_host(pc, ic)
        # iitro1: [G, 2, NFREE] channel-major i_it*RO
        iitro1 = np.ascontiguousarray(
            (pc * RO).reshape(G, GP, P, 2).transpose(0, 3, 1, 2)
        ).reshape(G, 2, NFREE).astype(np.float32)
        rocan1 = np.ascontiguousarray(
            (cc * RO).reshape(G, GP, P, 2).transpose(0, 3, 1, 2)
        ).reshape(G, 2, NFREE).astype(bf)
        bhw = np.broadcast_to(
            (ic.astype(np.float32) * HWPIX)[None, :], (P, NPC)
        ).copy()
        im = {
            "feat": feat_np,
            "offs1": offs1,
            "wts1": wts1,
            "iitro1": iitro1,
            "rocan1": rocan1,
            "bhw": bhw,
        }
        for s in range(3):
            wsd = wts_by_snake[s]
            im[f"w{s}_head"] = wsd["head"]
            for i in range(7):
                im[f"w{s}_res{i}"] = wsd["res"][i]
            im[f"w{s}_fus"] = wsd["fus"]
            im[f"w{s}_p0g"] = wsd["p0g"]
            im[f"w{s}_p0s"] = wsd["p0s"]
            im[f"w{s}_p1"] = wsd["p1"]
            im[f"w{s}_p2"] = wsd["p2"]
        in_maps.append(im)
    return in_maps


def _assemble_out(per_core_pyout):
    out = np.zeros((3, N, P, 2), np.float32)
    for core in range(NCORES):
        po = np.asarray(per_core_pyout[core], np.float32)  # [3, G, P, 8]
        po = po.reshape(3, G, P, GP, 2).transpose(0, 1, 3, 2, 4).reshape(3, NPC, P, 2)
        out[:, core * NPC : (core + 1) * NPC] = po
    return out


def kernel(cnn_feature, img_init_polys, can_init_polys, params, ind):
    cnn_feature = _np32(cnn_feature)
    img_init_polys = _np32(img_init_polys)
    can_init_polys = _np32(can_init_polys)
    ind = np.asarray(ind, dtype=np.int32)

    if "nc" not in _CACHE:
        _CACHE["nc"] = _build_program()
    nc = _CACHE["nc"]

    in_maps = _prepare_in_maps(cnn_feature, img_init_polys, can_init_polys, params, ind)
    res = run_bass_kernel_spmd(nc, in_maps, list(range(NCORES)))
    return _assemble_out([res.results[c]["pyout"] for c in range(NCORES)])
